# revision 2
# baseline (speedup 1.0000x reference)
"""DenseMaxPooling Trainium2 kernel.

reference: out = elementwise max over jnp.roll(x, s, axis=-1) for
s in {0, L/4, L/2, 3L/4}, x of shape (4096, 8192) f32.

Because the shifts are exactly L/4 apart, out[i, j] depends only on
(i, j mod L/4):  out[i, j] = max_k x[i, (j mod 2048) + k*2048].
So per row we max-reduce the 4 column blocks into a 2048-wide tile,
replicate it across the full row width in SBUF, and store the row
contiguously.  Pure memory-bound streaming: 16 MiB read + 16 MiB write
per core against a measured ~345-350 GB/s/core direction-agnostic DMA
ceiling, so the schedule's only job is to keep aggregate DMA at that
ceiling with minimal ramp.

Sharding: rows (dim 0) split evenly across 8 cores; no communication.

Schedule (per core, 4 row-tiles of 128 rows, all 4 tiles SBUF-resident
— no buffer reuse, so no cross-tile pipeline hazards):
  sync/SP   (HWDGE q1): load tiles 0,2   [128, 8192] -> xt0, xt2
  scalar/Act(HWDGE q2): load tiles 1,3   [128, 8192] -> xt1, xt3
  vector (DVE): per tile, a=max(q0,q1), b=max(q2,q3), q0=max(a,b),
                then copy q0 into q1..q3 (replicate in place)
  gpsimd/Pool (SWDGE): store tile t after its DVE pass; first store
                gated until tiles 0 and 1 have landed (gate=2: lets the
                two read queues establish before the write stream joins;
                measured optimum of the gate sweep g0..g4)
Splitting loads across the two HWDGE queues + stores on the SWDGE queue
measured ~5% faster than the single-load-queue variant (99.0us vs
104.1us per pass in drift-controlled interleaved A/Bs; 2-queue stores
and 64-row tiles both measured slower).

Written in raw Bass (manual semaphores): the walrus build in this
toolchain rejects any instruction carrying more than one sync wait, so
every wait sits on its own sequencer instruction.
"""

from contextlib import ExitStack

import numpy as np

import concourse.bass as bass
import concourse.mybir as mybir
from concourse.bass_utils import run_bass_kernel_spmd

N_CORES = 8
ROWS = 4096          # full input rows
L = 8192             # row length
Q = L // 4           # 2048, the period of the output
R = ROWS // N_CORES  # 512 rows per core
P = 128              # SBUF partitions (rows per tile)
NT = R // P          # 4 row-tiles per core

_NC_CACHE = {}


def build_nc():
    nc = bass.Bass()
    x = nc.declare_dram_parameter("x", [R, L], mybir.dt.float32, isOutput=False)
    y = nc.declare_dram_parameter("y", [R, L], mybir.dt.float32, isOutput=True)

    # tile t -> load queue: even tiles on SP, odd tiles on Act
    t_sp = [t for t in range(NT) if t % 2 == 0]
    t_act = [t for t in range(NT) if t % 2 == 1]

    with ExitStack() as st:
        xt = [
            st.enter_context(nc.sbuf_tensor(f"xt{t}", [P, L], mybir.dt.float32))
            for t in range(NT)
        ]
        a = st.enter_context(nc.sbuf_tensor("a", [P, Q], mybir.dt.float32))
        b = st.enter_context(nc.sbuf_tensor("b", [P, Q], mybir.dt.float32))
        ld_sp = st.enter_context(nc.semaphore("ld_sp"))
        ld_act = st.enter_context(nc.semaphore("ld_act"))
        cmp = st.enter_context(nc.semaphore("cmp"))
        stq = st.enter_context(nc.semaphore("stq"))
        block = st.enter_context(nc.Block())

        @block.sync
        def _(sy):
            for t in t_sp:
                sy.dma_start(xt[t][:], x[t * P:(t + 1) * P, :]).then_inc(
                    ld_sp, 16)

        @block.scalar
        def _(sc):
            for t in t_act:
                sc.dma_start(xt[t][:], x[t * P:(t + 1) * P, :]).then_inc(
                    ld_act, 16)

        @block.vector
        def _(vec):
            sp_seen = act_seen = 0
            for t in range(NT):
                if t % 2 == 0:
                    sp_seen += 16
                    vec.wait_ge(ld_sp, sp_seen)
                else:
                    act_seen += 16
                    vec.wait_ge(ld_act, act_seen)
                xi = xt[t]
                nc.vector.tensor_max(a[:], xi[:, 0:Q], xi[:, Q:2 * Q])
                nc.vector.tensor_max(b[:], xi[:, 2 * Q:3 * Q],
                                     xi[:, 3 * Q:4 * Q])
                nc.vector.tensor_max(xi[:, 0:Q], a[:], b[:])
                for k in range(1, 3):
                    nc.vector.tensor_copy(xi[:, k * Q:(k + 1) * Q], xi[:, 0:Q])
                nc.vector.tensor_copy(xi[:, 3 * Q:4 * Q],
                                      xi[:, 0:Q]).then_inc(cmp, 16)

        @block.gpsimd
        def _(gp):
            # gate=2: first store waits for tiles 0 (SP) and 1 (Act)
            gp.wait_ge(ld_sp, 16)
            gp.wait_ge(ld_act, 16)
            for t in range(NT):
                gp.wait_ge(cmp, 16 * (t + 1))
                gp.dma_start(y[t * P:(t + 1) * P, :], xt[t][:]).then_inc(
                    stq, 16)
            gp.wait_ge(stq, 16 * NT)

    return nc


def _get_nc():
    if "nc" not in _NC_CACHE:
        _NC_CACHE["nc"] = build_nc()
    return _NC_CACHE["nc"]


def kernel(inputs: np.ndarray) -> np.ndarray:
    x = np.ascontiguousarray(inputs, dtype=np.float32)
    assert x.shape == (ROWS, L), x.shape
    nc = _get_nc()
    in_maps = [{"x": x[i * R:(i + 1) * R]} for i in range(N_CORES)]
    res = run_bass_kernel_spmd(nc, in_maps, list(range(N_CORES)))
    return np.concatenate([res.results[i]["y"] for i in range(N_CORES)], axis=0)
